# revision 7
# baseline (speedup 1.0000x reference)
"""Trainium2 Bass kernel for DeformRoIPooling (nn_DeformRoIPooling_14516989461142).

Strategy:
  Host (cheap metadata only, derived from rois/offset [256x5, 256x2x7x7]):
    - replicate reference math in f32 to get, per roi, the 49*16 bilinear
      samples' 4 corner pixel indices and weights (valid mask and 1/count
      normalization folded into the weights)
    - dedup to per-roi unique pixel list (window) + dense weight matrix
      W[window_pixel, 49 bins]
    - sort rois by window size, deal round-robin over the 8 cores so the
      single SPMD program's per-slot padding is minimal
  Device (per core, 32 roi slots):
    - data replicated as NHWC [B*H*W, C] rows; per slot, indirect-DMA gather
      of the window pixel rows (128 rows x 1KB per K-tile)
    - PE matmul psum[49,256] += W_tile[128,49].T @ win_tile[128,256]
      accumulated over K-tiles
    - copy psum -> SBUF out buffer; one 1.6MB output DMA at the end
  Host: reassemble [256,256,7,7] from the 8 per-core outputs.
"""

import os

import numpy as np

# ---------------------------------------------------------------- constants
SPATIAL_SCALE = 0.125
POOLED = 7
SAMPLES = 4
TRANS_STD = 0.1
B, C, H, W = 4, 256, 64, 64
N_ROIS = 256
N_CORES = 8
NSLOT = N_ROIS // N_CORES
NBINS = POOLED * POOLED
F32 = np.float32

# matmul operand dtype mode: "f32" (safe) or "f32r" (full-rate fp32)
MM_MODE = os.environ.get("DRP_MM_MODE", "f32r")


# ------------------------------------------------------- host-side metadata
def _sample_weights(rois: np.ndarray, offset: np.ndarray):
    """Per roi: flat corner pixel indices [N, 49*16*4] and weights (f32)."""
    N = rois.shape[0]
    P, S = POOLED, SAMPLES
    b = rois[:, 0].astype(np.int32)
    roi_sw = (np.round(rois[:, 1]) * F32(SPATIAL_SCALE) - F32(0.5)).astype(F32)
    roi_sh = (np.round(rois[:, 2]) * F32(SPATIAL_SCALE) - F32(0.5)).astype(F32)
    roi_ew = ((np.round(rois[:, 3]) + F32(1.0)) * F32(SPATIAL_SCALE) - F32(0.5)).astype(F32)
    roi_eh = ((np.round(rois[:, 4]) + F32(1.0)) * F32(SPATIAL_SCALE) - F32(0.5)).astype(F32)
    roi_w = np.maximum(roi_ew - roi_sw, F32(0.1)).astype(F32)
    roi_h = np.maximum(roi_eh - roi_sh, F32(0.1)).astype(F32)
    bin_w = (roi_w / F32(P)).astype(F32)
    bin_h = (roi_h / F32(P)).astype(F32)
    sub_w = (bin_w / F32(S)).astype(F32)
    sub_h = (bin_h / F32(S)).astype(F32)
    ph = np.arange(P, dtype=F32)
    pw = np.arange(P, dtype=F32)
    tx = (offset[:, 0] * F32(TRANS_STD)).astype(F32)  # [N,P,P] (part == identity)
    ty = (offset[:, 1] * F32(TRANS_STD)).astype(F32)
    wstart = (pw[None, None, :] * bin_w[:, None, None] + roi_sw[:, None, None]
              + tx * roi_w[:, None, None]).astype(F32)
    hstart = (ph[None, :, None] * bin_h[:, None, None] + roi_sh[:, None, None]
              + ty * roi_h[:, None, None]).astype(F32)
    s = np.arange(S, dtype=F32)
    w = (wstart[..., None, None]
         + s[None, None, None, None, :] * sub_w[:, None, None, None, None]).astype(F32)
    h = (hstart[..., None, None]
         + s[None, None, None, :, None] * sub_h[:, None, None, None, None]).astype(F32)
    valid = (w > F32(-0.5)) & (w < F32(W - 0.5)) & (h > F32(-0.5)) & (h < F32(H - 0.5))
    wc = np.clip(w, F32(0.0), F32(W - 1.0)).astype(F32)
    hc = np.clip(h, F32(0.0), F32(H - 1.0)).astype(F32)
    x0 = np.floor(wc).astype(np.int32)
    y0 = np.floor(hc).astype(np.int32)
    x1 = np.minimum(x0 + 1, W - 1)
    y1 = np.minimum(y0 + 1, H - 1)
    dx = (wc - x0.astype(F32)).astype(F32)
    dy = (hc - y0.astype(F32)).astype(F32)
    count = valid.sum(axis=(3, 4)).astype(F32)
    norm = (F32(1.0) / np.maximum(count, F32(1.0))).astype(F32)
    vn = (valid.astype(F32) * norm[..., None, None]).astype(F32)
    k00 = ((F32(1.0) - dx) * (F32(1.0) - dy) * vn).astype(F32)
    k01 = ((F32(1.0) - dx) * dy * vn).astype(F32)
    k10 = (dx * (F32(1.0) - dy) * vn).astype(F32)
    k11 = (dx * dy * vn).astype(F32)
    base = (b * (H * W)).astype(np.int32)[:, None, None, None, None]
    f00 = base + y0 * W + x0
    f01 = base + y1 * W + x0
    f10 = base + y0 * W + x1
    f11 = base + y1 * W + x1
    flat = np.stack([f00, f01, f10, f11], axis=-1).reshape(N, -1)
    wts = np.stack([k00, k01, k10, k11], axis=-1).reshape(N, -1).astype(F32)
    return flat, wts


def _per_roi_tables(rois, offset):
    flat, wts = _sample_weights(rois, offset)
    per_entry_bin = np.repeat(np.arange(NBINS), SAMPLES * SAMPLES * 4)
    tables = []
    for n in range(rois.shape[0]):
        uniq, inv = np.unique(flat[n], return_inverse=True)
        Wn = np.zeros((len(uniq), NBINS), dtype=np.float64)
        np.add.at(Wn, (inv, per_entry_bin), wts[n].astype(np.float64))
        tables.append((uniq.astype(np.int32), Wn.astype(F32)))
    return tables


def _plan(rois, offset):
    """Build the per-core packed idx/weight arrays and the slot layout."""
    tables = _per_roi_tables(rois, offset)
    sizes = np.array([len(u) for u, _ in tables])
    order = np.argsort(-sizes, kind="stable")  # big rois first
    # slot j on core k processes roi order[j*8+k]
    nblk = np.empty(NSLOT, dtype=np.int64)
    for j in range(NSLOT):
        mx = max(sizes[order[j * N_CORES + k]] for k in range(N_CORES))
        nblk[j] = max(1, -(-int(mx) // 128))
    cofs = np.concatenate([[0], np.cumsum(nblk)])[:NSLOT + 1]
    ncol = int(cofs[-1])
    idx_np = np.zeros((N_CORES, 128, ncol), dtype=np.int32)
    w_np = np.zeros((N_CORES, 128, ncol * NBINS), dtype=F32)
    for j in range(NSLOT):
        for k in range(N_CORES):
            n = order[j * N_CORES + k]
            uniq, Wn = tables[n]
            K = len(uniq)
            kp = int(nblk[j]) * 128
            upad = np.zeros(kp, dtype=np.int32)
            upad[:K] = uniq
            wpad = np.zeros((kp, NBINS), dtype=F32)
            wpad[:K] = Wn
            for t in range(int(nblk[j])):
                c = int(cofs[j]) + t
                idx_np[k, :, c] = upad[t * 128:(t + 1) * 128]
                w_np[k, :, c * NBINS:(c + 1) * NBINS] = wpad[t * 128:(t + 1) * 128]
    return order, nblk, cofs, ncol, idx_np, w_np


# ------------------------------------------------------------ device kernel
def _build_nc(nblk, cofs, ncol):
    import concourse.bass as bass
    import concourse.mybir as mybir
    import concourse.tile as tile
    from concourse import bacc
    from concourse._compat import get_trn_type

    nc = bacc.Bacc(get_trn_type() or "TRN2", target_bir_lowering=False)
    mm_dt_in = (mybir.dt.float32r if MM_MODE == "f32r" else mybir.dt.float32)
    dsrc = nc.dram_tensor("dsrc", [B * H * W, C], mm_dt_in,
                          kind="ExternalInput")
    idx_d = nc.dram_tensor("idx", [128, ncol], mybir.dt.int32,
                           kind="ExternalInput")
    w_d = nc.dram_tensor("wmat", [128, ncol * NBINS], mm_dt_in,
                         kind="ExternalInput")
    out_d = nc.dram_tensor("out", [NBINS, NSLOT * C], mybir.dt.float32,
                           kind="ExternalOutput")

    mm_dt = mybir.dt.float32r if MM_MODE == "f32r" else mybir.dt.float32

    with tile.TileContext(nc) as tc:
        with (
            tc.tile_pool(name="const", bufs=1) as cpool,
            tc.tile_pool(name="win", bufs=6) as wpool,
            tc.tile_pool(name="psum", bufs=6, space="PSUM") as ppool,
            tc.tile_pool(name="outp", bufs=1) as opool,
        ):
            idx_t = cpool.tile([128, ncol], mybir.dt.int32)
            nc.sync.dma_start(idx_t[:], idx_d[:])
            w_t = cpool.tile([128, ncol * NBINS], mm_dt)
            nc.sync.dma_start(w_t[:], w_d[:])
            out_t = opool.tile([NBINS, NSLOT * C], mybir.dt.float32)
            for j in range(NSLOT):
                nb = int(nblk[j])
                co = int(cofs[j])
                win = wpool.tile([128, nb * C], mm_dt, tag="win")
                for t in range(nb):
                    nc.gpsimd.indirect_dma_start(
                        out=win[:, t * C:(t + 1) * C],
                        out_offset=None,
                        in_=dsrc[:],
                        in_offset=bass.IndirectOffsetOnAxis(
                            ap=idx_t[:, co + t:co + t + 1], axis=0),
                    )
                ps = ppool.tile([NBINS, C], mybir.dt.float32, space="PSUM")
                for t in range(nb):
                    c = co + t
                    nc.tensor.matmul(out=ps[:],
                                     lhsT=w_t[:, c * NBINS:(c + 1) * NBINS],
                                     rhs=win[:, t * C:(t + 1) * C],
                                     start=(t == 0), stop=(t == nb - 1))
                nc.vector.tensor_copy(out=out_t[:, j * C:(j + 1) * C], in_=ps[:])
            nc.sync.dma_start(out_d[:], out_t[:])
    nc.compile()
    return nc


# ------------------------------------------------------------------- driver
def kernel(data: np.ndarray, rois: np.ndarray, offset: np.ndarray,
           _trace: bool = False):
    from concourse.bass_utils import run_bass_kernel_spmd

    data = np.ascontiguousarray(data, dtype=F32)
    rois = np.asarray(rois, dtype=F32)
    offset = np.asarray(offset, dtype=F32)

    order, nblk, cofs, ncol, idx_np, w_np = _plan(rois, offset)
    dsrc = np.ascontiguousarray(data.transpose(0, 2, 3, 1)).reshape(B * H * W, C)

    nc = _build_nc(nblk, cofs, ncol)
    in_maps = [
        {"dsrc": dsrc, "idx": idx_np[k], "wmat": w_np[k]}
        for k in range(N_CORES)
    ]
    res = run_bass_kernel_spmd(nc, in_maps, core_ids=list(range(N_CORES)),
                               trace=_trace)

    out = np.zeros((N_ROIS, C, POOLED, POOLED), dtype=F32)
    for k in range(N_CORES):
        oc = res.results[k]["out"]  # [49, NSLOT*C]
        for j in range(NSLOT):
            n = order[j * N_CORES + k]
            out[n] = oc[:, j * C:(j + 1) * C].T.reshape(C, POOLED, POOLED)
    if _trace:
        kernel.last_results = res
    return out


# revision 8
# speedup vs baseline: 1.0246x; 1.0246x over previous
"""Trainium2 Bass kernel for DeformRoIPooling (nn_DeformRoIPooling_14516989461142).

Strategy:
  Host (cheap metadata only, derived from rois/offset):
    - replicate reference math in f32 to get, per roi, the 49*16 bilinear
      samples' 4 corner pixel indices and weights (valid mask and 1/count
      normalization folded into the weights)
    - dedup to per-roi unique pixel list (window) + dense weight matrix
      W[window_pixel, 49 bins]
    - sort rois by window size, deal round-robin over the 8 cores so the
      single SPMD program's per-slot padding is minimal
  Device (per core, 32 roi slots):
    - data replicated as NHWC [B*H*W, C] rows; per slot, gpsimd dma_gather
      of the window pixel rows (K x 1KB) into K-tiles [128, 256]
    - PE matmul psum[49,256] += W_tile[128,49].T @ win_tile[128,256]
      accumulated over K-tiles (float32r = full-rate fp32)
    - DVE copy psum -> SBUF out buffer (roi pairs packed at partition 0/64),
      output DMA'd out in chunks overlapping the loop
  Host: reassemble [256,256,7,7] from the 8 per-core outputs.
"""

import os

import numpy as np

# ---------------------------------------------------------------- constants
SPATIAL_SCALE = 0.125
POOLED = 7
SAMPLES = 4
TRANS_STD = 0.1
B, C, H, W = 4, 256, 64, 64
N_ROIS = 256
N_CORES = 8
NSLOT = N_ROIS // N_CORES
NBINS = POOLED * POOLED
F32 = np.float32
OUT_CHUNK = 8  # slots per output-DMA chunk (pairs: OUT_CHUNK/2 cols of 256)

# matmul operand dtype mode: "f32" (safe) or "f32r" (full-rate fp32)
MM_MODE = os.environ.get("DRP_MM_MODE", "f32r")


# ------------------------------------------------------- host-side metadata
def _sample_weights(rois: np.ndarray, offset: np.ndarray):
    """Per roi: flat corner pixel indices [N, 49*16*4] and weights (f32)."""
    N = rois.shape[0]
    P, S = POOLED, SAMPLES
    b = rois[:, 0].astype(np.int32)
    roi_sw = (np.round(rois[:, 1]) * F32(SPATIAL_SCALE) - F32(0.5)).astype(F32)
    roi_sh = (np.round(rois[:, 2]) * F32(SPATIAL_SCALE) - F32(0.5)).astype(F32)
    roi_ew = ((np.round(rois[:, 3]) + F32(1.0)) * F32(SPATIAL_SCALE) - F32(0.5)).astype(F32)
    roi_eh = ((np.round(rois[:, 4]) + F32(1.0)) * F32(SPATIAL_SCALE) - F32(0.5)).astype(F32)
    roi_w = np.maximum(roi_ew - roi_sw, F32(0.1)).astype(F32)
    roi_h = np.maximum(roi_eh - roi_sh, F32(0.1)).astype(F32)
    bin_w = (roi_w / F32(P)).astype(F32)
    bin_h = (roi_h / F32(P)).astype(F32)
    sub_w = (bin_w / F32(S)).astype(F32)
    sub_h = (bin_h / F32(S)).astype(F32)
    ph = np.arange(P, dtype=F32)
    pw = np.arange(P, dtype=F32)
    tx = (offset[:, 0] * F32(TRANS_STD)).astype(F32)  # [N,P,P] (part == identity)
    ty = (offset[:, 1] * F32(TRANS_STD)).astype(F32)
    wstart = (pw[None, None, :] * bin_w[:, None, None] + roi_sw[:, None, None]
              + tx * roi_w[:, None, None]).astype(F32)
    hstart = (ph[None, :, None] * bin_h[:, None, None] + roi_sh[:, None, None]
              + ty * roi_h[:, None, None]).astype(F32)
    s = np.arange(S, dtype=F32)
    w = (wstart[..., None, None]
         + s[None, None, None, None, :] * sub_w[:, None, None, None, None]).astype(F32)
    h = (hstart[..., None, None]
         + s[None, None, None, :, None] * sub_h[:, None, None, None, None]).astype(F32)
    valid = (w > F32(-0.5)) & (w < F32(W - 0.5)) & (h > F32(-0.5)) & (h < F32(H - 0.5))
    wc = np.clip(w, F32(0.0), F32(W - 1.0)).astype(F32)
    hc = np.clip(h, F32(0.0), F32(H - 1.0)).astype(F32)
    x0 = np.floor(wc).astype(np.int32)
    y0 = np.floor(hc).astype(np.int32)
    x1 = np.minimum(x0 + 1, W - 1)
    y1 = np.minimum(y0 + 1, H - 1)
    dx = (wc - x0.astype(F32)).astype(F32)
    dy = (hc - y0.astype(F32)).astype(F32)
    count = valid.sum(axis=(3, 4)).astype(F32)
    norm = (F32(1.0) / np.maximum(count, F32(1.0))).astype(F32)
    vn = (valid.astype(F32) * norm[..., None, None]).astype(F32)
    k00 = ((F32(1.0) - dx) * (F32(1.0) - dy) * vn).astype(F32)
    k01 = ((F32(1.0) - dx) * dy * vn).astype(F32)
    k10 = (dx * (F32(1.0) - dy) * vn).astype(F32)
    k11 = (dx * dy * vn).astype(F32)
    base = (b * (H * W)).astype(np.int32)[:, None, None, None, None]
    f00 = base + y0 * W + x0
    f01 = base + y1 * W + x0
    f10 = base + y0 * W + x1
    f11 = base + y1 * W + x1
    flat = np.stack([f00, f01, f10, f11], axis=-1).reshape(N, -1)
    wts = np.stack([k00, k01, k10, k11], axis=-1).reshape(N, -1).astype(F32)
    return flat, wts


def _per_roi_tables(rois, offset):
    flat, wts = _sample_weights(rois, offset)
    per_entry_bin = np.repeat(np.arange(NBINS), SAMPLES * SAMPLES * 4)
    tables = []
    for n in range(rois.shape[0]):
        uniq, inv = np.unique(flat[n], return_inverse=True)
        Wn = np.zeros((len(uniq), NBINS), dtype=np.float64)
        np.add.at(Wn, (inv, per_entry_bin), wts[n].astype(np.float64))
        tables.append((uniq.astype(np.int32), Wn.astype(F32)))
    return tables


def _plan(rois, offset):
    """Build the per-core packed idx/weight arrays and the slot layout."""
    tables = _per_roi_tables(rois, offset)
    sizes = np.array([len(u) for u, _ in tables])
    order = np.argsort(-sizes, kind="stable")  # big rois first
    # slot j on core k processes roi order[j*8+k]
    nblk = np.empty(NSLOT, dtype=np.int64)
    for j in range(NSLOT):
        mx = max(sizes[order[j * N_CORES + k]] for k in range(N_CORES))
        nblk[j] = max(1, -(-int(mx) // 128))
    cofs = np.concatenate([[0], np.cumsum(nblk)])[:NSLOT + 1]
    ncol = int(cofs[-1])
    # idx: int16, dma_gather wrapped layout. slot j occupies columns
    # [8*cofs[j], 8*cofs[j+1]): index i of the slot at [i%16, 8*cofs[j]+i//16],
    # replicated across the 8 16-partition groups.
    idx_np = np.zeros((N_CORES, 16, ncol * 8), dtype=np.int16)
    w_np = np.zeros((N_CORES, 128, ncol * NBINS), dtype=F32)
    for j in range(NSLOT):
        kp = int(nblk[j]) * 128
        for k in range(N_CORES):
            n = order[j * N_CORES + k]
            uniq, Wn = tables[n]
            K = len(uniq)
            upad = np.zeros(kp, dtype=np.int16)
            upad[:K] = uniq
            wpad = np.zeros((kp, NBINS), dtype=F32)
            wpad[:K] = Wn
            c16 = int(cofs[j]) * 8
            idx_np[k, :, c16:c16 + kp // 16] = upad.reshape(kp // 16, 16).T
            for t in range(int(nblk[j])):
                c = int(cofs[j]) + t
                w_np[k, :, c * NBINS:(c + 1) * NBINS] = wpad[t * 128:(t + 1) * 128]
    idx_np = np.tile(idx_np, (1, 8, 1))  # replicate to 128 partitions
    return order, nblk, cofs, ncol, idx_np, w_np


# ------------------------------------------------------------ device kernel
def _build_nc(nblk, cofs, ncol):
    import concourse.mybir as mybir
    import concourse.tile as tile
    from concourse import bacc
    from concourse._compat import get_trn_type

    nc = bacc.Bacc(get_trn_type() or "TRN2", target_bir_lowering=False)
    mm_dt = mybir.dt.float32r if MM_MODE == "f32r" else mybir.dt.float32
    dsrc = nc.dram_tensor("dsrc", [B * H * W, C], mm_dt, kind="ExternalInput")
    idx_d = nc.dram_tensor("idx", [128, ncol * 8], mybir.dt.int16,
                           kind="ExternalInput")
    w_d = nc.dram_tensor("wmat", [128, ncol * NBINS], mm_dt,
                         kind="ExternalInput")
    npair_cols = (NSLOT // 2) * C
    out_d = nc.dram_tensor("out", [128, npair_cols], mybir.dt.float32,
                           kind="ExternalOutput")

    with tile.TileContext(nc) as tc:
        with (
            tc.tile_pool(name="const", bufs=1) as cpool,
            tc.tile_pool(name="win", bufs=6) as wpool,
            tc.tile_pool(name="psum", bufs=6, space="PSUM") as ppool,
            tc.tile_pool(name="outp", bufs=1) as opool,
        ):
            idx_t = cpool.tile([128, ncol * 8], mybir.dt.int16)
            nc.sync.dma_start(idx_t[:], idx_d[:])
            w_t = cpool.tile([128, ncol * NBINS], mm_dt)
            nc.sync.dma_start(w_t[:], w_d[:])
            out_t = opool.tile([128, npair_cols], mybir.dt.float32)
            for j in range(NSLOT):
                nb = int(nblk[j])
                co = int(cofs[j])
                K = nb * 128
                win = wpool.tile([128, nb * C], mm_dt, tag="win")
                win3 = win[:].rearrange("p (b e) -> p b e", e=C)
                nc.gpsimd.dma_gather(
                    win3, dsrc[:], idx_t[:, co * 8:co * 8 + K // 16],
                    K, K, C)
                ps = ppool.tile([NBINS, C], mybir.dt.float32, space="PSUM")
                for t in range(nb):
                    c = co + t
                    nc.tensor.matmul(out=ps[:],
                                     lhsT=w_t[:, c * NBINS:(c + 1) * NBINS],
                                     rhs=win[:, t * C:(t + 1) * C],
                                     start=(t == 0), stop=(t == nb - 1))
                po = 0 if j % 2 == 0 else 64
                col = (j // 2) * C
                nc.vector.tensor_copy(out=out_t[po:po + NBINS, col:col + C],
                                      in_=ps[:])
                if j % OUT_CHUNK == OUT_CHUNK - 1:
                    c0 = (j // OUT_CHUNK) * (OUT_CHUNK // 2) * C
                    c1 = c0 + (OUT_CHUNK // 2) * C
                    nc.sync.dma_start(out_d[:, c0:c1], out_t[:, c0:c1])
    nc.compile()
    return nc


# ------------------------------------------------------------------- driver
def kernel(data: np.ndarray, rois: np.ndarray, offset: np.ndarray,
           _trace: bool = False):
    from concourse.bass_utils import run_bass_kernel_spmd

    data = np.ascontiguousarray(data, dtype=F32)
    rois = np.asarray(rois, dtype=F32)
    offset = np.asarray(offset, dtype=F32)

    order, nblk, cofs, ncol, idx_np, w_np = _plan(rois, offset)
    dsrc = np.ascontiguousarray(data.transpose(0, 2, 3, 1)).reshape(B * H * W, C)

    nc = _build_nc(nblk, cofs, ncol)
    in_maps = [
        {"dsrc": dsrc, "idx": idx_np[k], "wmat": w_np[k]}
        for k in range(N_CORES)
    ]
    res = run_bass_kernel_spmd(nc, in_maps, core_ids=list(range(N_CORES)),
                               trace=_trace)

    out = np.zeros((N_ROIS, C, POOLED, POOLED), dtype=F32)
    for k in range(N_CORES):
        oc = res.results[k]["out"]  # [128, (NSLOT//2)*C]
        for j in range(NSLOT):
            n = order[j * N_CORES + k]
            po = 0 if j % 2 == 0 else 64
            col = (j // 2) * C
            out[n] = oc[po:po + NBINS, col:col + C].T.reshape(C, POOLED, POOLED)
    if _trace:
        kernel.last_results = res
    return out
